# revision 13
# baseline (speedup 1.0000x reference)
"""TP(heads)xDP(batch) sharded causal GQA attention block for 8 trn2 cores.

Each core c handles batch b=c//4 and head group g=c%4 (8 q heads, 2 kv heads).
Per-core pipeline (fused over 4 query chunks of 512):
  qkv = Wqkv_c @ x_b^T  (bf16 matmuls, outputs [feature, token] layout)
  RoPE on q/k (DVE + stream_shuffle partition swap)
  scores_T[kj, qi] = k^T q  (f32r, 2 heads packed in the 128-row PE array)
  exp on ScalarE (no max subtraction; scores are ~N(0,1) after 1/8 scaling)
  out_aug[65, qi] = [v; ones]^T exp  (bf16; row 64 = softmax denominator)
  normalize via DVE reciprocal + gpsimd partition_broadcast
  partial_T[dout, t] = Wo_c^T attn  (bf16), host sums the 8 partials.
"""
import sys
sys.path.insert(0, "/opt/trn_rl_repo")
from contextlib import ExitStack

import numpy as np
import ml_dtypes

B, L, D = 2, 2048, 2048
NH, NKV, HD = 32, 8, 64
ROPE_BASE = 10000.0
SCALE = HD ** -0.5
TC, TCW = 4, 512      # query/token chunks
NKD = 16              # d contraction tiles
NOT = 6               # output tiles per core (4 q packs, k pack, v pack)
NKJ = 16              # key tiles

F16 = ml_dtypes.bfloat16

_cached = {}


def _build_nc():
    import concourse.bacc as bacc
    import concourse.tile as tile
    import concourse.mybir as mybir
    from concourse import library_config

    F32 = mybir.dt.float32
    F32R = mybir.dt.float32r
    BF = mybir.dt.bfloat16
    AF = mybir.ActivationFunctionType

    nc = bacc.Bacc("TRN2", debug=False)
    xh_ap = nc.dram_tensor("xh", (TC, 128, NKD * TCW), BF, kind="ExternalInput").ap()
    wq_ap = nc.dram_tensor("wq", (128, NOT * NKD * 128), BF, kind="ExternalInput").ap()
    wo_ap = nc.dram_tensor("wo", (128, 4 * NKD * 128), BF, kind="ExternalInput").ap()
    ccss_ap = nc.dram_tensor("ccss", (128, 2 * L), F32, kind="ExternalInput").ap()
    msk_ap = nc.dram_tensor("msk", (128, 4 * TCW), BF, kind="ExternalInput").ap()
    id2_ap = nc.dram_tensor("id2", (128, 64), F32R, kind="ExternalInput").ap()
    prm_ap = nc.dram_tensor("prm", (128, 128), BF, kind="ExternalInput").ap()
    out_ap = nc.dram_tensor("outp", (TC, NKD, 128, TCW), BF, kind="ExternalOutput").ap()

    with tile.TileContext(nc) as tcx, ExitStack() as ctx:
        pc = ctx.enter_context(tcx.tile_pool(name="const", bufs=1))
        px = ctx.enter_context(tcx.tile_pool(name="x", bufs=2))
        pw = ctx.enter_context(tcx.tile_pool(name="work", bufs=1))
        psc = ctx.enter_context(tcx.tile_pool(name="psc", bufs=2, space="PSUM"))
        paug = ctx.enter_context(tcx.tile_pool(name="paug", bufs=1, space="PSUM"))
        pmm = ctx.enter_context(tcx.tile_pool(name="pmm", bufs=2, space="PSUM"))

        wq_t = pc.tile([128, NOT * NKD * 128], BF)
        wo_t = pc.tile([128, 4 * NKD * 128], BF)
        ccss_t = pc.tile([128, 2 * L], F32)
        msk_t = pc.tile([128, 4 * TCW], BF)
        id2_t = pc.tile([128, 64], F32R)
        prm_t = pc.tile([128, 128], BF)
        wqw = NKD * 128
        # priority order: k-weights + rope tables first so chunk-0 rope/scores
        # start while the rest of the weights stream in.
        nc.sync.dma_start(wq_t[:, 4 * wqw:5 * wqw], wq_ap[:, 4 * wqw:5 * wqw])
        nc.sync.dma_start(ccss_t[:, 0:TCW], ccss_ap[:, 0:TCW])
        nc.sync.dma_start(ccss_t[:, L:L + TCW], ccss_ap[:, L:L + TCW])
        nc.sync.dma_start(prm_t[:, :], prm_ap[:, :])
        nc.sync.dma_start(wq_t[:, 5 * wqw:6 * wqw], wq_ap[:, 5 * wqw:6 * wqw])
        nc.sync.dma_start(id2_t[:, :], id2_ap[:, :])
        for ot in (0, 1, 2, 3):
            nc.sync.dma_start(wq_t[:, ot * wqw:(ot + 1) * wqw],
                              wq_ap[:, ot * wqw:(ot + 1) * wqw])
        nc.sync.dma_start(msk_t[:, :], msk_ap[:, :])
        nc.sync.dma_start(ccss_t[:, TCW:L], ccss_ap[:, TCW:L])
        nc.sync.dma_start(ccss_t[:, L + TCW:2 * L], ccss_ap[:, L + TCW:2 * L])
        wow = 4 * NKD * 128 // 4
        for i in range(4):
            nc.sync.dma_start(wo_t[:, i * wow:(i + 1) * wow], wo_ap[:, i * wow:(i + 1) * wow])

        kpack = pc.tile([128, L], BF)        # k (2 kv heads stacked), RoPE'd
        vaug = pc.tile([128, 2 * NKJ * 65], BF)  # [v | ones] per (kv, kj)
        nc.vector.memset(vaug[:, 64::65], 1.0)   # ones columns

        with tcx.tile_critical():
            nc.gpsimd.load_library(library_config.attn)

        _pairs = [None] * 4

        def rope_one(raw, tci, r):
            cs = ccss_t[:, tci * TCW:(tci + 1) * TCW]
            ss = ccss_t[:, L + tci * TCW:L + (tci + 1) * TCW]
            sl = slice(r * TCW, (r + 1) * TCW)
            swp = pmm.tile([128, TCW], F32, tag="mm")
            nc.tensor.matmul(swp[:, :], prm_t[:, :], raw[:, sl],
                             start=True, stop=True)
            with tcx.high_priority():
                nc.vector.tensor_mul(swp[:, :], swp[:, :], ss)
                nc.vector.tensor_mul(raw[:, sl], raw[:, sl], cs)
                nc.vector.tensor_add(raw[:, sl], raw[:, sl], swp[:, :])

        def rope_batched(raw, dest_ap, tci, nrep):
            """dest = raw*CC + swap32(raw)*SS; raw is fp16 [128, nrep*TCW] sbuf.
            Partition swap comes from a PE permutation matmul (prm_t)."""
            cs = ccss_t[:, tci * TCW:(tci + 1) * TCW]
            ss = ccss_t[:, L + tci * TCW:L + (tci + 1) * TCW]
            for r in range(nrep):
                sl = slice(r * TCW, (r + 1) * TCW)
                swp = pmm.tile([128, TCW], F32, tag="mm")
                nc.tensor.matmul(swp[:, :], prm_t[:, :], raw[:, sl],
                                 start=True, stop=True)
                with tcx.high_priority():
                    nc.vector.tensor_mul(swp[:, :], swp[:, :], ss)
                    nc.vector.tensor_mul(raw[:, sl], raw[:, sl], cs)
                    nc.vector.tensor_add(dest_ap[:, sl], raw[:, sl], swp[:, :])

        def o_proj_tiles(otc, dts, pairs_):
            for dt in dts:
                po = pmm.tile([128, TCW], F32, tag="mm")
                for kt in range(4):
                    nc.tensor.matmul(
                        po[:, :], wo_t[:, (kt * NKD + dt) * 128:(kt * NKD + dt + 1) * 128],
                        pairs_[kt][:, :],
                        start=(kt == 0), stop=(kt == 3))
                ev = pw.tile([128, TCW], BF, tag="ev", bufs=2)
                nc.vector.tensor_copy(ev[:, :], po[:, :])
                nc.sync.dma_start(out_ap[otc, dt], ev[:, :])

        def emit_xt_dma(tci):
            xt = px.tile([128, NKD * TCW], BF, tag="xt")
            xw = NKD * TCW // 8
            with tcx.high_priority():
                for i in range(8):
                    nc.sync.dma_start(xt[:, i * xw:(i + 1) * xw],
                                      xh_ap[tci][:, i * xw:(i + 1) * xw])
            return xt

        def qkv_ot(tci, xt, qraw, ot):
            ps = pmm.tile([128, TCW], F32, tag="mm")
            for dt in range(NKD):
                nc.tensor.matmul(
                    ps[:, :], wq_t[:, (ot * NKD + dt) * 128:(ot * NKD + dt + 1) * 128],
                    xt[:, dt * TCW:(dt + 1) * TCW],
                    start=(dt == 0), stop=(dt == NKD - 1))
            if ot == 4:
                kraw = pw.tile([128, TCW], BF, tag="kraw", bufs=2)
                with tcx.high_priority():
                    nc.vector.tensor_copy(kraw[:, :], ps[:, :])
                rope_batched(kraw, kpack[:, tci * TCW:(tci + 1) * TCW], tci, 1)
            elif ot == 5:
                vch = pw.tile([128, TCW], F32R, tag="vch", bufs=2)
                with tcx.high_priority():
                    nc.vector.tensor_copy(vch[:, :], ps[:, :])
                for j in range(2):
                    for jj in range(4):
                        kj = 4 * tci + jj
                        tp = pmm.tile([128, 64], F32R, tag="mm")
                        nc.tensor.transpose(
                            tp[:, :], vch[64 * j:64 * j + 64, jj * 128:(jj + 1) * 128],
                            id2_t[64 * j:64 * j + 64, :])
                        col = (j * NKJ + kj) * 65
                        with tcx.high_priority():
                            nc.vector.tensor_copy(vaug[:, col:col + 64], tp[:, :])
            else:
                with tcx.high_priority():
                    nc.vector.tensor_copy(qraw[:, ot * TCW:(ot + 1) * TCW], ps[:, :])
                rope_one(qraw, tci, ot)

        # ---- prologue: chunk 0 qkv ----
        xt_cur = emit_xt_dma(0)
        qraw_cur = pw.tile([128, 4 * TCW], BF, tag="qraw", bufs=2)
        for ot in (4, 5, 0, 1, 2, 3):
            qkv_ot(0, xt_cur, qraw_cur, ot)

        QKV_SLOTS = ((4,), (5,), (0, 1), (2, 3))
        prev_pairs = None
        for tci in range(TC):
            tcs = slice(tci * TCW, (tci + 1) * TCW)
            qall = qraw_cur
            nxt = tci + 1
            if nxt < TC:
                xt_nxt = emit_xt_dma(nxt)
                qraw_nxt = pw.tile([128, 4 * TCW], BF, tag="qraw", bufs=2)

            # ---- attention for query chunk tci, all 4 packs ----
            augs = [None] * 4
            last_kj = 4 * tci + 3
            for p in range(4):
                qs = slice(p * TCW, (p + 1) * TCW)
                augA = paug.tile([65, TCW], F32, tag="augA")
                augB = paug.tile([65, TCW], F32, tag="augB")
                for kj in range(4 * tci + 4):
                    ks = slice(kj * 128, (kj + 1) * 128)
                    dj = kj - 4 * tci
                    # causal trim: query cols < 128*dj of this chunk see no
                    # key of tile kj, so skip them in scores/exp/AV entirely.
                    qc0 = dj * 128 if dj > 0 else 0
                    qsA = slice(p * TCW + qc0, (p + 1) * TCW)
                    scp = psc.tile([128, 2, TCW], F32, tag="scp")
                    nc.tensor.matmul(scp[:, 0, qc0:TCW], kpack[0:64, ks], qall[0:64, qsA],
                                     start=True, stop=True, tile_position=(0, 0))
                    nc.tensor.matmul(scp[:, 1, qc0:TCW], kpack[64:128, ks], qall[64:128, qsA],
                                     start=True, stop=True, tile_position=(64, 0))
                    ep = pw.tile([128, 2, TCW], BF, tag="ep", bufs=6)
                    nc.scalar.activation(ep[:, :, qc0:TCW], scp[:, :, qc0:TCW],
                                         AF.Exp, scale=SCALE)
                    if dj >= 0:
                        mb = (msk_t[:, 0:128].unsqueeze(1)
                              .broadcast_to([128, 2, 128]))
                        with tcx.high_priority():
                            nc.vector.tensor_mul(ep[:, :, qc0:qc0 + 128],
                                                 ep[:, :, qc0:qc0 + 128], mb)
                    colA = (0 * NKJ + kj) * 65
                    colB = (1 * NKJ + kj) * 65
                    nc.tensor.matmul(augA[:, qc0:TCW], vaug[:, colA:colA + 65],
                                     ep[:, 0, qc0:TCW],
                                     start=(kj == 0), stop=(kj == last_kj))
                    nc.tensor.matmul(augB[:, qc0:TCW], vaug[:, colB:colB + 65],
                                     ep[:, 1, qc0:TCW],
                                     start=(kj == 0), stop=(kj == last_kj))
                # evacuate psum quickly so the next pack's accumulators start
                augS = pw.tile([65, 2, TCW], F32, tag="augS", bufs=4)
                with tcx.high_priority():
                    nc.vector.tensor_copy(augS[:, 0, :], augA[:, :])
                    nc.vector.tensor_copy(augS[:, 1, :], augB[:, :])
                # normalization for this pack: batched den hop + reciprocal +
                # gpsimd partition broadcast (pair has a full chunk of slack)
                den2 = pw.tile([2, TCW], F32, tag="den2", bufs=4)
                nc.gpsimd.dma_start(den2[:, :], augS[64:65, :, :])
                rc2 = pw.tile([2, TCW], F32, tag="rc2", bufs=4)
                nc.vector.reciprocal_approx_fast(rc2[:, :], den2[:, :])
                rcp = pw.tile([1, 2 * TCW], F32, tag="rcp", bufs=4)
                nc.gpsimd.dma_start(rcp[:, :], rc2[:, :])
                pair = pw.tile([128, TCW], BF, tag="pair", bufs=8)
                bA = pw.tile([64, TCW], F32, tag="bA", bufs=2)
                nc.gpsimd.partition_broadcast(bA[:, :], rcp[0:1, 0:TCW])
                nc.vector.tensor_mul(pair[0:64, :], augS[0:64, 0, :], bA[:, :])
                bB = pw.tile([64, TCW], F32, tag="bB", bufs=2)
                nc.gpsimd.partition_broadcast(bB[:, :], rcp[0:1, TCW:2 * TCW])
                ob = pw.tile([64, TCW], BF, tag="ob", bufs=3)
                nc.vector.tensor_mul(ob[:, :], augS[0:64, 1, :], bB[:, :])
                nc.gpsimd.dma_start(pair[64:128, :], ob[:, :])
                _pairs[p] = pair
                # PE filler for the ACT-bound kj loop: o_proj of the previous
                # chunk and qkv of the next chunk, one slice per pack slot.
                if tci >= 1:
                    o_proj_tiles(tci - 1, range(4 * p, 4 * p + 4), prev_pairs)
                if nxt < TC:
                    for ot in QKV_SLOTS[p]:
                        qkv_ot(nxt, xt_nxt, qraw_nxt, ot)

            prev_pairs = list(_pairs)
            if nxt < TC:
                qraw_cur = qraw_nxt
                xt_cur = xt_nxt

        for p in range(4):
            o_proj_tiles(TC - 1, range(4 * p, 4 * p + 4), prev_pairs)

    nc.compile()
    return nc


def _host_prep(x, Wqkv, Wo):
    """Build per-core input maps. Returns list of 8 dicts."""
    invfreq = 1.0 / (ROPE_BASE ** (np.arange(0, HD, 2, dtype=np.float32) / HD))
    ang = np.arange(L, dtype=np.float32)[:, None] * invfreq[None, :]   # [L, 32]
    cos = np.cos(ang).T     # [32, L]
    sin = np.sin(ang).T
    cc = np.tile(cos, (4, 1)).astype(np.float32)                       # [128, L]
    sgn = np.repeat(np.array([-1.0, 1.0, -1.0, 1.0], np.float32), 32)
    ss = (np.tile(sin, (4, 1)) * sgn[:, None]).astype(np.float32)
    ccss = np.concatenate([cc, ss], axis=1)                            # [128, 2L]

    r = np.arange(128)[:, None]
    c = np.arange(TCW)[None, :]
    msk = np.concatenate(
        [(r + 128 * j <= c).astype(np.float32) for j in range(4)], axis=1
    ).astype(F16)                                                      # [128, 2048]

    id2 = np.zeros((128, 64), np.float32)
    id2[:64] = np.eye(64, dtype=np.float32)
    id2[64:] = np.eye(64, dtype=np.float32)

    prm = np.zeros((128, 128), np.float32)
    prm[np.arange(128), np.arange(128) ^ 32] = 1.0                     # swap32 perm
    prm = prm.astype(F16)

    wq_part = Wqkv[:NH * HD].reshape(NH, HD, D)
    wk_part = Wqkv[NH * HD:NH * HD + NKV * HD].reshape(NKV, HD, D)
    wv_part = Wqkv[NH * HD + NKV * HD:].reshape(NKV, HD, D)

    in_maps = []
    for core in range(8):
        b, g = core // 4, core % 4
        xT = np.ascontiguousarray(x[b].T)                              # [D, L]
        xh = (xT.reshape(NKD, 128, TC, TCW).transpose(2, 1, 0, 3)
              .reshape(TC, 128, NKD * TCW)).astype(F16)

        rows = []
        for p in range(4):
            rows.append(wq_part[8 * g + p])
            rows.append(wq_part[8 * g + 4 + p])
        rows.append(wk_part[2 * g]); rows.append(wk_part[2 * g + 1])
        rows.append(wv_part[2 * g]); rows.append(wv_part[2 * g + 1])
        Wc = np.concatenate(rows, axis=0)                              # [768, D]
        wq = (Wc.reshape(NOT, 128, NKD, 128).transpose(3, 0, 2, 1)
              .reshape(128, NOT * NKD * 128)).astype(F16)

        cols = np.empty((4, 128), np.int64)
        for kt in range(4):
            cols[kt, :64] = (8 * g + kt) * HD + np.arange(64)
            cols[kt, 64:] = (8 * g + 4 + kt) * HD + np.arange(64)
        Woc = Wo.T[cols.reshape(-1)]                                   # [512, D]
        wo = (Woc.reshape(4, 128, NKD, 128).transpose(1, 0, 2, 3)
              .reshape(128, 4 * NKD * 128)).astype(F16)

        in_maps.append(dict(xh=xh, wq=wq, wo=wo, ccss=ccss, msk=msk, id2=id2, prm=prm))
    return in_maps


def _get_nc():
    if "nc" not in _cached:
        _cached["nc"] = _build_nc()
    return _cached["nc"]


def run_sharded(x, Wqkv, Wo, trace=False):
    """Run on 8 cores; returns (out [B,L,D] float32, BassKernelResults)."""
    from concourse.bass_utils import run_bass_kernel_spmd
    nc = _get_nc()
    in_maps = _host_prep(np.asarray(x, np.float32), np.asarray(Wqkv, np.float32),
                         np.asarray(Wo, np.float32))
    res = run_bass_kernel_spmd(nc, in_maps, list(range(8)), trace=trace)
    out = np.zeros((B, L, D), np.float64)
    for core in range(8):
        b = core // 4
        P = res.results[core]["outp"].transpose(1, 2, 0, 3).reshape(D, L)
        out[b] += P.T.astype(np.float64)
    return out.astype(np.float32), res


def kernel(x, Wqkv, Wo):
    out, _ = run_sharded(x, Wqkv, Wo, trace=False)
    return out



# revision 14
# speedup vs baseline: 1.1858x; 1.1858x over previous
"""TP(heads)xDP(batch) sharded causal GQA attention block for 8 trn2 cores.

Each core c handles batch b=c//4 and head group g=c%4 (8 q heads, 2 kv heads).
Per-core pipeline (fused over 4 query chunks of 512):
  qkv = Wqkv_c @ x_b^T  (bf16 matmuls, outputs [feature, token] layout)
  RoPE on q/k (DVE + stream_shuffle partition swap)
  scores_T[kj, qi] = k^T q  (f32r, 2 heads packed in the 128-row PE array)
  exp on ScalarE (no max subtraction; scores are ~N(0,1) after 1/8 scaling)
  out_aug[65, qi] = [v; ones]^T exp  (bf16; row 64 = softmax denominator)
  normalize via DVE reciprocal + gpsimd partition_broadcast
  partial_T[dout, t] = Wo_c^T attn  (bf16), host sums the 8 partials.
"""
import sys
sys.path.insert(0, "/opt/trn_rl_repo")
from contextlib import ExitStack

import numpy as np
import ml_dtypes

B, L, D = 2, 2048, 2048
NH, NKV, HD = 32, 8, 64
ROPE_BASE = 10000.0
SCALE = HD ** -0.5
TC, TCW = 4, 512      # query/token chunks
NKD = 16              # d contraction tiles
NOT = 6               # output tiles per core (4 q packs, k pack, v pack)
NKJ = 16              # key tiles

F16 = ml_dtypes.bfloat16

_cached = {}


def _build_nc():
    import concourse.bacc as bacc
    import concourse.tile as tile
    import concourse.mybir as mybir
    from concourse import library_config

    F32 = mybir.dt.float32
    F32R = mybir.dt.float32r
    BF = mybir.dt.bfloat16
    AF = mybir.ActivationFunctionType

    nc = bacc.Bacc("TRN2", debug=False)
    xh_ap = nc.dram_tensor("xh", (TC, 128, NKD * TCW), BF, kind="ExternalInput").ap()
    wq_ap = nc.dram_tensor("wq", (128, NOT * NKD * 128), BF, kind="ExternalInput").ap()
    wo_ap = nc.dram_tensor("wo", (128, 4 * NKD * 128), BF, kind="ExternalInput").ap()
    ccss_ap = nc.dram_tensor("ccss", (128, 2 * L), F32, kind="ExternalInput").ap()
    msk_ap = nc.dram_tensor("msk", (128, 4 * TCW), BF, kind="ExternalInput").ap()
    id2_ap = nc.dram_tensor("id2", (128, 64), F32R, kind="ExternalInput").ap()
    prm_ap = nc.dram_tensor("prm", (128, 128), BF, kind="ExternalInput").ap()
    out_ap = nc.dram_tensor("outp", (TC, NKD, 128, TCW), BF, kind="ExternalOutput").ap()

    with tile.TileContext(nc) as tcx, ExitStack() as ctx:
        pc = ctx.enter_context(tcx.tile_pool(name="const", bufs=1))
        px = ctx.enter_context(tcx.tile_pool(name="x", bufs=2))
        pw = ctx.enter_context(tcx.tile_pool(name="work", bufs=1))
        psc = ctx.enter_context(tcx.tile_pool(name="psc", bufs=2, space="PSUM"))
        paug = ctx.enter_context(tcx.tile_pool(name="paug", bufs=1, space="PSUM"))
        pmm = ctx.enter_context(tcx.tile_pool(name="pmm", bufs=2, space="PSUM"))

        wq_t = pc.tile([128, NOT * NKD * 128], BF)
        wo_t = pc.tile([128, 4 * NKD * 128], BF)
        ccss_t = pc.tile([128, 2 * L], F32)
        msk_t = pc.tile([128, 4 * TCW], BF)
        id2_t = pc.tile([128, 64], F32R)
        prm_t = pc.tile([128, 128], BF)
        wqw = NKD * 128
        # priority order: k-weights + rope tables first so chunk-0 rope/scores
        # start while the rest of the weights stream in.
        nc.sync.dma_start(wq_t[:, 4 * wqw:5 * wqw], wq_ap[:, 4 * wqw:5 * wqw])
        nc.sync.dma_start(ccss_t[:, 0:TCW], ccss_ap[:, 0:TCW])
        nc.sync.dma_start(ccss_t[:, L:L + TCW], ccss_ap[:, L:L + TCW])
        nc.sync.dma_start(prm_t[:, :], prm_ap[:, :])
        nc.sync.dma_start(wq_t[:, 5 * wqw:6 * wqw], wq_ap[:, 5 * wqw:6 * wqw])
        nc.sync.dma_start(id2_t[:, :], id2_ap[:, :])
        for ot in (0, 1, 2, 3):
            nc.sync.dma_start(wq_t[:, ot * wqw:(ot + 1) * wqw],
                              wq_ap[:, ot * wqw:(ot + 1) * wqw])
        nc.sync.dma_start(msk_t[:, :], msk_ap[:, :])
        nc.sync.dma_start(ccss_t[:, TCW:L], ccss_ap[:, TCW:L])
        nc.sync.dma_start(ccss_t[:, L + TCW:2 * L], ccss_ap[:, L + TCW:2 * L])
        wow = 4 * NKD * 128 // 4
        for i in range(4):
            nc.sync.dma_start(wo_t[:, i * wow:(i + 1) * wow], wo_ap[:, i * wow:(i + 1) * wow])

        kpack = pc.tile([128, L], BF)        # k (2 kv heads stacked), RoPE'd
        vaug = pc.tile([128, 2 * NKJ * 65], BF)  # [v | ones] per (kv, kj)
        nc.vector.memset(vaug[:, 64::65], 1.0)   # ones columns

        with tcx.tile_critical():
            nc.gpsimd.load_library(library_config.attn)

        _pairs = [None] * 4

        def rope_one(raw, tci, r):
            cs = ccss_t[:, tci * TCW:(tci + 1) * TCW]
            ss = ccss_t[:, L + tci * TCW:L + (tci + 1) * TCW]
            sl = slice(r * TCW, (r + 1) * TCW)
            swp = pmm.tile([128, TCW], F32, tag="mm")
            nc.tensor.matmul(swp[:, :], prm_t[:, :], raw[:, sl],
                             start=True, stop=True)
            with tcx.high_priority():
                nc.vector.tensor_mul(swp[:, :], swp[:, :], ss)
                nc.vector.tensor_mul(raw[:, sl], raw[:, sl], cs)
                nc.vector.tensor_add(raw[:, sl], raw[:, sl], swp[:, :])

        def rope_batched(raw, dest_ap, tci, nrep):
            """dest = raw*CC + swap32(raw)*SS; raw is fp16 [128, nrep*TCW] sbuf.
            Partition swap comes from a PE permutation matmul (prm_t)."""
            cs = ccss_t[:, tci * TCW:(tci + 1) * TCW]
            ss = ccss_t[:, L + tci * TCW:L + (tci + 1) * TCW]
            for r in range(nrep):
                sl = slice(r * TCW, (r + 1) * TCW)
                swp = pmm.tile([128, TCW], F32, tag="mm")
                nc.tensor.matmul(swp[:, :], prm_t[:, :], raw[:, sl],
                                 start=True, stop=True)
                with tcx.high_priority():
                    nc.vector.tensor_mul(swp[:, :], swp[:, :], ss)
                    nc.vector.tensor_mul(raw[:, sl], raw[:, sl], cs)
                    nc.vector.tensor_add(dest_ap[:, sl], raw[:, sl], swp[:, :])

        def o_proj_tiles(otc, dts, pairs_):
            for dt in dts:
                po = pmm.tile([128, TCW], F32, tag="mm")
                for kt in range(4):
                    nc.tensor.matmul(
                        po[:, :], wo_t[:, (kt * NKD + dt) * 128:(kt * NKD + dt + 1) * 128],
                        pairs_[kt][:, :],
                        start=(kt == 0), stop=(kt == 3))
                ev = pw.tile([128, TCW], BF, tag="ev", bufs=2)
                nc.vector.tensor_copy(ev[:, :], po[:, :])
                nc.sync.dma_start(out_ap[otc, dt], ev[:, :])

        def emit_xt_dma(tci):
            xt = px.tile([128, NKD * TCW], BF, tag="xt")
            xw = NKD * TCW // 8
            with tcx.high_priority():
                for i in range(8):
                    nc.sync.dma_start(xt[:, i * xw:(i + 1) * xw],
                                      xh_ap[tci][:, i * xw:(i + 1) * xw])
            return xt

        def qkv_ot(tci, xt, qraw, ot):
            ps = pmm.tile([128, TCW], F32, tag="mm")
            for dt in range(NKD):
                nc.tensor.matmul(
                    ps[:, :], wq_t[:, (ot * NKD + dt) * 128:(ot * NKD + dt + 1) * 128],
                    xt[:, dt * TCW:(dt + 1) * TCW],
                    start=(dt == 0), stop=(dt == NKD - 1))
            if ot == 4:
                kraw = pw.tile([128, TCW], BF, tag="kraw", bufs=2)
                with tcx.high_priority():
                    nc.vector.tensor_copy(kraw[:, :], ps[:, :])
                rope_batched(kraw, kpack[:, tci * TCW:(tci + 1) * TCW], tci, 1)
            elif ot == 5:
                vch = pw.tile([128, TCW], F32R, tag="vch", bufs=2)
                with tcx.high_priority():
                    nc.vector.tensor_copy(vch[:, :], ps[:, :])
                for j in range(2):
                    for jj in range(4):
                        kj = 4 * tci + jj
                        tp = pmm.tile([128, 64], F32R, tag="mm")
                        nc.tensor.transpose(
                            tp[:, :], vch[64 * j:64 * j + 64, jj * 128:(jj + 1) * 128],
                            id2_t[64 * j:64 * j + 64, :])
                        col = (j * NKJ + kj) * 65
                        with tcx.high_priority():
                            nc.vector.tensor_copy(vaug[:, col:col + 64], tp[:, :])
            else:
                with tcx.high_priority():
                    nc.vector.tensor_copy(qraw[:, ot * TCW:(ot + 1) * TCW], ps[:, :])
                rope_one(qraw, tci, ot)

        # ---- prologue: chunk 0 qkv ----
        xt_cur = emit_xt_dma(0)
        qraw_cur = pw.tile([128, 4 * TCW], BF, tag="qraw", bufs=2)
        for ot in (4, 5, 0, 1, 2, 3):
            qkv_ot(0, xt_cur, qraw_cur, ot)

        QKV_SLOTS = ((4,), (5,), (0, 1), (2, 3))
        prev_pairs = None
        for tci in range(TC):
            tcs = slice(tci * TCW, (tci + 1) * TCW)
            qall = qraw_cur
            nxt = tci + 1
            if nxt < TC:
                xt_nxt = emit_xt_dma(nxt)
                qraw_nxt = pw.tile([128, 4 * TCW], BF, tag="qraw", bufs=2)

            # ---- attention for query chunk tci, all 4 packs ----
            augs = [None] * 4
            last_kj = 4 * tci + 3
            for p in range(4):
                qs = slice(p * TCW, (p + 1) * TCW)
                augA = paug.tile([65, TCW], F32, tag="augA")
                augB = paug.tile([65, TCW], F32, tag="augB")
                for kj in range(4 * tci + 4):
                    ks = slice(kj * 128, (kj + 1) * 128)
                    dj = kj - 4 * tci
                    # causal trim: query cols < 128*dj of this chunk see no
                    # key of tile kj, so skip them in scores/exp/AV entirely.
                    qc0 = dj * 128 if dj > 0 else 0
                    qsA = slice(p * TCW + qc0, (p + 1) * TCW)
                    scp = psc.tile([128, 2, TCW], F32, tag="scp")
                    nc.tensor.matmul(scp[:, 0, qc0:TCW], kpack[0:64, ks], qall[0:64, qsA],
                                     start=True, stop=True, tile_position=(0, 0))
                    nc.tensor.matmul(scp[:, 1, qc0:TCW], kpack[64:128, ks], qall[64:128, qsA],
                                     start=True, stop=True, tile_position=(64, 0))
                    ep = pw.tile([128, 2, TCW], BF, tag="ep", bufs=5)
                    nc.scalar.activation(ep[:, :, qc0:TCW], scp[:, :, qc0:TCW],
                                         AF.Exp, scale=SCALE)
                    if dj >= 0:
                        mb = (msk_t[:, 0:128].unsqueeze(1)
                              .broadcast_to([128, 2, 128]))
                        with tcx.high_priority():
                            nc.vector.tensor_mul(ep[:, :, qc0:qc0 + 128],
                                                 ep[:, :, qc0:qc0 + 128], mb)
                    colA = (0 * NKJ + kj) * 65
                    colB = (1 * NKJ + kj) * 65
                    nc.tensor.matmul(augA[:, qc0:TCW], vaug[:, colA:colA + 65],
                                     ep[:, 0, qc0:TCW],
                                     start=(kj == 0), stop=(kj == last_kj))
                    nc.tensor.matmul(augB[:, qc0:TCW], vaug[:, colB:colB + 65],
                                     ep[:, 1, qc0:TCW],
                                     start=(kj == 0), stop=(kj == last_kj))
                # evacuate psum quickly so the next pack's accumulators start
                augS = pw.tile([65, 2, TCW], F32, tag="augS", bufs=4)
                with tcx.high_priority():
                    nc.vector.tensor_copy(augS[:, 0, :], augA[:, :])
                    nc.vector.tensor_copy(augS[:, 1, :], augB[:, :])
                # normalization for this pack: batched den hop + reciprocal +
                # gpsimd partition broadcast (pair has a full chunk of slack)
                den2 = pw.tile([2, TCW], F32, tag="den2", bufs=4)
                nc.sync.dma_start(den2[:, :], augS[64:65, :, :])
                rc2 = pw.tile([2, TCW], F32, tag="rc2", bufs=4)
                nc.vector.reciprocal_approx_fast(rc2[:, :], den2[:, :])
                rcp = pw.tile([1, 2 * TCW], F32, tag="rcp", bufs=4)
                nc.sync.dma_start(rcp[:, :], rc2[:, :])
                pair = pw.tile([128, TCW], BF, tag="pair", bufs=8)
                bA = pw.tile([64, TCW], F32, tag="bA", bufs=2)
                nc.gpsimd.partition_broadcast(bA[:, :], rcp[0:1, 0:TCW])
                nc.vector.tensor_mul(pair[0:64, :], augS[0:64, 0, :], bA[:, :])
                bB = pw.tile([64, TCW], F32, tag="bB", bufs=2)
                nc.gpsimd.partition_broadcast(bB[:, :], rcp[0:1, TCW:2 * TCW])
                ob = pw.tile([64, TCW], BF, tag="ob", bufs=3)
                nc.vector.tensor_mul(ob[:, :], augS[0:64, 1, :], bB[:, :])
                nc.sync.dma_start(pair[64:128, :], ob[:, :])
                _pairs[p] = pair
                # PE filler for the ACT-bound kj loop: o_proj of the previous
                # chunk and qkv of the next chunk, one slice per pack slot.
                if tci >= 1:
                    o_proj_tiles(tci - 1, range(4 * p, 4 * p + 4), prev_pairs)
                if nxt < TC:
                    for ot in QKV_SLOTS[p]:
                        qkv_ot(nxt, xt_nxt, qraw_nxt, ot)

            prev_pairs = list(_pairs)
            if nxt < TC:
                qraw_cur = qraw_nxt
                xt_cur = xt_nxt

        for p in range(4):
            o_proj_tiles(TC - 1, range(4 * p, 4 * p + 4), prev_pairs)

    nc.compile()
    return nc


def _host_prep(x, Wqkv, Wo):
    """Build per-core input maps. Returns list of 8 dicts."""
    invfreq = 1.0 / (ROPE_BASE ** (np.arange(0, HD, 2, dtype=np.float32) / HD))
    ang = np.arange(L, dtype=np.float32)[:, None] * invfreq[None, :]   # [L, 32]
    cos = np.cos(ang).T     # [32, L]
    sin = np.sin(ang).T
    cc = np.tile(cos, (4, 1)).astype(np.float32)                       # [128, L]
    sgn = np.repeat(np.array([-1.0, 1.0, -1.0, 1.0], np.float32), 32)
    ss = (np.tile(sin, (4, 1)) * sgn[:, None]).astype(np.float32)
    ccss = np.concatenate([cc, ss], axis=1)                            # [128, 2L]

    r = np.arange(128)[:, None]
    c = np.arange(TCW)[None, :]
    msk = np.concatenate(
        [(r + 128 * j <= c).astype(np.float32) for j in range(4)], axis=1
    ).astype(F16)                                                      # [128, 2048]

    id2 = np.zeros((128, 64), np.float32)
    id2[:64] = np.eye(64, dtype=np.float32)
    id2[64:] = np.eye(64, dtype=np.float32)

    prm = np.zeros((128, 128), np.float32)
    prm[np.arange(128), np.arange(128) ^ 32] = 1.0                     # swap32 perm
    prm = prm.astype(F16)

    wq_part = Wqkv[:NH * HD].reshape(NH, HD, D)
    wk_part = Wqkv[NH * HD:NH * HD + NKV * HD].reshape(NKV, HD, D)
    wv_part = Wqkv[NH * HD + NKV * HD:].reshape(NKV, HD, D)

    in_maps = []
    for core in range(8):
        b, g = core // 4, core % 4
        xT = np.ascontiguousarray(x[b].T)                              # [D, L]
        xh = (xT.reshape(NKD, 128, TC, TCW).transpose(2, 1, 0, 3)
              .reshape(TC, 128, NKD * TCW)).astype(F16)

        rows = []
        for p in range(4):
            rows.append(wq_part[8 * g + p])
            rows.append(wq_part[8 * g + 4 + p])
        rows.append(wk_part[2 * g]); rows.append(wk_part[2 * g + 1])
        rows.append(wv_part[2 * g]); rows.append(wv_part[2 * g + 1])
        Wc = np.concatenate(rows, axis=0)                              # [768, D]
        wq = (Wc.reshape(NOT, 128, NKD, 128).transpose(3, 0, 2, 1)
              .reshape(128, NOT * NKD * 128)).astype(F16)

        cols = np.empty((4, 128), np.int64)
        for kt in range(4):
            cols[kt, :64] = (8 * g + kt) * HD + np.arange(64)
            cols[kt, 64:] = (8 * g + 4 + kt) * HD + np.arange(64)
        Woc = Wo.T[cols.reshape(-1)]                                   # [512, D]
        wo = (Woc.reshape(4, 128, NKD, 128).transpose(1, 0, 2, 3)
              .reshape(128, 4 * NKD * 128)).astype(F16)

        in_maps.append(dict(xh=xh, wq=wq, wo=wo, ccss=ccss, msk=msk, id2=id2, prm=prm))
    return in_maps


def _get_nc():
    if "nc" not in _cached:
        _cached["nc"] = _build_nc()
    return _cached["nc"]


def run_sharded(x, Wqkv, Wo, trace=False):
    """Run on 8 cores; returns (out [B,L,D] float32, BassKernelResults)."""
    from concourse.bass_utils import run_bass_kernel_spmd
    nc = _get_nc()
    in_maps = _host_prep(np.asarray(x, np.float32), np.asarray(Wqkv, np.float32),
                         np.asarray(Wo, np.float32))
    res = run_bass_kernel_spmd(nc, in_maps, list(range(8)), trace=trace)
    out = np.zeros((B, L, D), np.float64)
    for core in range(8):
        b = core // 4
        P = res.results[core]["outp"].transpose(1, 2, 0, 3).reshape(D, L)
        out[b] += P.T.astype(np.float64)
    return out.astype(np.float32), res


def kernel(x, Wqkv, Wo):
    out, _ = run_sharded(x, Wqkv, Wo, trace=False)
    return out

